# revision 14
# baseline (speedup 1.0000x reference)
"""Trainium2 Bass kernel for a 2-layer GCN (PyG GCNConv + dense layer).

Computation (matches the jax reference):
    deg[n]  = 1 + sum of incoming edge weights        (self loop weight 1)
    dinv    = deg ** -0.5
    norm_e  = dinv[src] * ew * dinv[dst]              (per edge, incl. self)
    agg[n]  = sum_e norm_e * x[src_e]                 (propagate FIRST: A(xW) == (Ax)W)
    h       = relu(agg @ W1 + b1)
    out     = relu(h @ W2 + b2)

Distribution: nodes (as scatter destinations) are partitioned across the 8
cores.  All on-chip compute is bf16 (rel err ~4e-3 vs the 2e-2 gate).

Normalization is folded into host-built tables so the device only does
gather + one-hot matmuls + dense layers:
    xtab[n]   = dinv[n] * x[n]       (bf16 gather table, even nodes first)
    w_e       = ew_e * dinv[dst_e]   (per-edge S value)
    xpermT[n] = dinv[n]^2 * x[n]     (self-loop term, added at eviction)

Per core, nodes are degree-sorted into 128-node dst tiles.  Each tile's
incoming edges are stored ELL-style: ELL chunk c holds the c-th edge of
every dst position, so the tile's S block is a run of diagonals generated
by ONE DVE scalar_tensor_tensor over stride-0 broadcast APs; leftover
high-degree edges go to compact one-hot chunks (iota==mdst S, one STT
each).  Gathered rows are node-major (dma_gather from HBM in bf16, 256B
descriptors; -1 pad indices skip their descriptor), and each 128-edge
chunk feeds  lhsT^T @ S  accumulated feature-major in PSUM.  Self loops
are never gathered: the xpermT table is added during the PSUM->SBUF
eviction (scalar_tensor_tensor add).  Dense W1/W2 run feature-major in
bf16; the output stays feature-major and contiguous, and the host does
the final transpose + row un-permutation.

Known HW constraints baked in: dma_gather max 1024 indices per
single-packet instruction; gathers rotate over 4 SWDGE queues; per-gather
touch matmuls keep S3_LW matmuls at <=2 sync waits; int16 gather indices
cap table views at 32768 rows (even/odd interleave keeps the two views
balanced); SPMD means one program serves all 8 cores, so chunk counts are
padded to the per-tile max across cores (pad idx = -1 -> descriptor
skipped, pad S value = 0).
"""

import os
import sys

import numpy as np

sys.path.insert(0, "/opt/trn_rl_repo")

P = 128
N_CORES = 8
HALF = 32768          # int16 index limit per gather table view
G_TILES = 4           # dst tiles per batch (one PSUM [128, 512] bank)
GMAX = 8              # chunks per gather instruction (1024 idx, 1 packet)

D_IN = 128
D_HID = 512
D_OUT = 128


def _best_k_shared(deg_list):
    """ELL depth K for one (tile, half) shared by all cores.  An ELL chunk
    costs a matmul + amortized slice of one wide STT + pad-slot DMA dups; an
    overflow chunk costs a matmul + its own full STT + DMA.  Minimize
    2*K + 3*max_core(ceil(overflow_c(K)/128)), ties toward smaller K."""
    dmax = max(int(d.max(initial=0)) for d in deg_list)
    if dmax == 0:
        return 0
    best = None
    for k in range(0, dmax + 1):
        novm = max(-(-int(np.maximum(d - k, 0).sum()) // P) for d in deg_list)
        key = (2 * k + 3 * novm, k)
        if best is None or key < best[0]:
            best = (key, k)
    return best[1]


def _preprocess(x, edge_index, edge_weight):
    """Graph preprocessing; per-core raw ELL/overflow structures."""
    N = x.shape[0]
    n_per = N // N_CORES
    assert n_per * N_CORES == N

    src = np.asarray(edge_index[0], np.int64)
    dst = np.asarray(edge_index[1], np.int64)
    ew = np.asarray(edge_weight, np.float32)
    ids = np.arange(N, dtype=np.int64)
    dst_f = np.concatenate([dst, ids])
    ew_f = np.concatenate([ew, np.ones(N, np.float32)])

    deg = np.bincount(dst_f, weights=ew_f.astype(np.float64), minlength=N)
    deg = deg.astype(np.float32)
    dinv = np.where(deg > 0, 1.0 / np.sqrt(deg), 0.0).astype(np.float32)

    # self loops are handled by the xpermT table; only real edges get slots
    w_all = (ew * dinv[dst]).astype(np.float32)

    interleave = N > HALF
    rows_a = (N + 1) // 2 if interleave else N

    n_tiles = -(-n_per // P)
    n_batches = -(-n_tiles // G_TILES)
    tiles_tot = n_batches * G_TILES

    per_core = []
    for c in range(N_CORES):
        lo = c * n_per
        m = (dst >= lo) & (dst < lo + n_per)
        es, ed, wc = src[m], (dst[m] - lo).astype(np.int64), w_all[m]
        if interleave:
            e_half = (es % 2).astype(np.int64)
            e_idx = (es // 2).astype(np.int64)
        else:
            e_half = np.zeros(len(es), np.int64)
            e_idx = es

        # degree-sorted dst tiles: uniform per-tile degree -> tight ELL.
        # Degree-rank groups are dealt round-robin across batches so every
        # batch carries a balanced chunk load (rank r -> batch r % n_batches).
        degl = np.bincount(ed, minlength=n_per)
        order_nodes = np.argsort(-degl, kind="stable")
        rank = np.arange(n_per) // P
        remap = np.empty(n_tiles, np.int64)
        for r in range(n_tiles):
            remap[r] = (r % n_batches) * G_TILES + (r // n_batches)
        tile_of = np.empty(n_per, np.int32)
        pos_in_tile = np.empty(n_per, np.int32)
        tile_of[order_nodes] = remap[rank]
        pos_in_tile[order_nodes] = np.arange(n_per) % P

        te = tile_of[ed]
        pe_ = pos_in_tile[ed]

        tiles = []
        for t in range(tiles_tot):
            th = {}
            for h in (0, 1):
                sel = (te == t) & (e_half == h)
                eposs, eidxs, ws = pe_[sel], e_idx[sel], wc[sel]
                o = np.lexsort((np.arange(len(eposs)), eposs))
                th[h] = dict(
                    degs=np.bincount(eposs, minlength=P),
                    eidxs=eidxs[o], ws=ws[o],
                )
            tiles.append(th)

        perm = np.full(tiles_tot * P, -1, np.int64)
        node_rows = tile_of.astype(np.int64) * P + pos_in_tile
        perm[node_rows] = np.arange(n_per) + lo

        per_core.append(dict(tiles=tiles, perm=perm, lo=lo,
                             tile_of=tile_of, pos_in_tile=pos_in_tile))

    layout = dict(
        n_batches=n_batches, tiles_tot=tiles_tot, n_tiles_real=n_tiles,
        n_rows_A=rows_a, n_rows_B=(N - rows_a) if interleave else 0,
        interleave=interleave,
    )
    return per_core, layout, dinv


def _build_ell(th, k, nov):
    """Materialize one (tile, half)'s ELL block at depth k plus nov overflow
    chunks.  Pad indices MUST stay valid (the HW gather treats every
    non-trailing index as an address); use the previous slot's index."""
    degs, eidxs, ws = th["degs"], th["eidxs"], th["ws"]
    starts = np.concatenate([[0], np.cumsum(degs)])
    ell_idx = np.full((k, P), -1, np.int64)
    ell_w = np.zeros((k, P), np.float32)
    ovf_i, ovf_p, ovf_w = [], [], []
    for p in range(P):
        s0, d = int(starts[p]), int(degs[p])
        take = min(d, k)
        ell_idx[:take, p] = eidxs[s0:s0 + take]
        ell_w[:take, p] = ws[s0:s0 + take]
        if d > k:
            ovf_i.append(eidxs[s0 + take:s0 + d])
            ovf_p.append(np.full(d - k, p, np.int64))
            ovf_w.append(ws[s0 + take:s0 + d])
    ovf_i = np.concatenate(ovf_i) if ovf_i else np.empty(0, np.int64)
    ovf_p = np.concatenate(ovf_p) if ovf_p else np.empty(0, np.int64)
    ovf_w = np.concatenate(ovf_w) if ovf_w else np.empty(0, np.float32)
    assert len(ovf_i) <= nov * P
    pad = nov * P - len(ovf_i)
    th["k"], th["nov"] = k, nov
    th["ell_idx"] = ell_idx
    th["ell_w"] = ell_w
    th["ovf_idx"] = np.concatenate(
        [ovf_i, np.full(pad, -1, np.int64)]).reshape(nov, P)
    th["ovf_pos"] = np.concatenate(
        [ovf_p, np.zeros(pad, np.int64)]).reshape(nov, P)
    th["ovf_w"] = np.concatenate(
        [ovf_w, np.zeros(pad, np.float32)]).reshape(nov, P)


def _schedule(per_core, layout):
    """Shared chunk/window schedule: every (tile, half) uses one ELL depth
    across all cores (chosen to minimize the shared padded chunk count), and
    overflow chunk counts are padded to the max across cores."""
    n_batches = layout["n_batches"]
    tiles_tot = layout["tiles_tot"]

    kpad = np.zeros((tiles_tot, 2), np.int64)
    novpad = np.zeros((tiles_tot, 2), np.int64)
    for t in range(tiles_tot):
        for h in (0, 1):
            k = _best_k_shared([pc["tiles"][t][h]["degs"] for pc in per_core])
            kpad[t, h] = k
            nov = 0
            for pc in per_core:
                degs = pc["tiles"][t][h]["degs"]
                nov = max(nov, -(-int(np.maximum(degs - k, 0).sum()) // P))
            novpad[t, h] = nov
            for pc in per_core:
                _build_ell(pc["tiles"][t][h], k, nov)
    ell_w = kpad.sum(axis=1)            # S_ell width per tile
    ovf_n = novpad.sum(axis=1)
    layout["kpad"] = kpad
    layout["novpad"] = novpad
    layout["ell_w"] = ell_w
    layout["ovf_n"] = ovf_n
    layout["kmax"] = int(max(1, ell_w.max()))

    # slot map per batch + window schedule; chunk stream per (batch, view):
    # for each tile of the batch: ELL chunks then ovf chunks
    win_sched = []          # (batch, view, [chunks per window])
    slot_maps = []          # per batch: {(t, "ell"/"ovf", combined_idx): slot}
    slots_max = 1
    for g in range(n_batches):
        smap = {}
        slot = 0
        for h in (0, 1):
            h0 = slot
            for t in range(g * G_TILES, (g + 1) * G_TILES):
                base_e = 0 if h == 0 else kpad[t, 0]
                base_o = 0 if h == 0 else novpad[t, 0]
                for cc in range(kpad[t, h]):
                    smap[(t, "ell", base_e + cc)] = slot
                    slot += 1
                for cc in range(novpad[t, h]):
                    smap[(t, "ovf", base_o + cc)] = slot
                    slot += 1
            n_ch = slot - h0
            wins = []
            while n_ch > 0:
                take = min(GMAX, n_ch)
                wins.append(take)
                n_ch -= take
            if wins:
                win_sched.append((g, h, wins))
        slot_maps.append(smap)
        slots_max = max(slots_max, slot)
    layout["win_sched"] = win_sched
    layout["slot_maps"] = slot_maps
    layout["slots_max"] = slots_max
    total_idx = sum(sum(w) for (_, _, ws) in win_sched for w in [ws]) * P
    layout["idx_cols"] = max(8, total_idx // 16)

    # cdata16 layout: iota(128) | pcol(1) | w1(512) | w2r(512) |
    #                 wELL blocks | (w, mdst) pairs per ovf chunk
    off = 1153
    O_WELL = []
    for t in range(tiles_tot):
        O_WELL.append(off)
        off += int(ell_w[t])
    O_OVF = []
    for t in range(tiles_tot):
        O_OVF.append(off)
        off += 2 * int(ovf_n[t])
    layout["O_WELL"] = O_WELL
    layout["O_OVF"] = O_OVF
    layout["C16"] = off
    return layout


def _build_program(layout):
    from concourse import bacc, mybir, tile

    f32 = mybir.dt.float32
    bf16 = mybir.dt.bfloat16
    i16 = mybir.dt.int16

    n_batches = layout["n_batches"]
    tiles_tot = layout["tiles_tot"]
    slots_max = layout["slots_max"]
    idx_cols = layout["idx_cols"]
    NA, NB = layout["n_rows_A"], layout["n_rows_B"]
    kmax = layout["kmax"]
    C16 = layout["C16"]
    O_WELL, O_OVF = layout["O_WELL"], layout["O_OVF"]
    ell_w, ovf_n = layout["ell_w"], layout["ovf_n"]
    O_IOTA, O_PCOL, O_W1, O_W2 = 0, 128, 129, 641
    O_B1, O_B2, C32 = 0, 4, 5

    # Gather wall: each 1024-idx dma_gather costs ~8.6us of Q7 exec on its
    # queue's cpu pair; 4 queues run pairs concurrently -> ~2.2us/instr
    # steady state.  Everything else must hide under that.
    nc = bacc.Bacc("TRN2", num_swdge_queues=4)
    xtab = nc.declare_dram_parameter("xtab", [NA + NB, D_IN], bf16,
                                     isOutput=False)
    xpermT_d = nc.declare_dram_parameter("xpermT", [P, tiles_tot * P], bf16,
                                         isOutput=False)
    c16_d = nc.declare_dram_parameter("cdata16", [P, C16], bf16,
                                      isOutput=False)
    c32_d = nc.declare_dram_parameter("cdata32", [P, C32], f32,
                                      isOutput=False)
    gidx_d = nc.declare_dram_parameter("gidx", [P, idx_cols], i16,
                                       isOutput=False)
    out_d = nc.declare_dram_parameter("out", [P, tiles_tot * P], f32,
                                      isOutput=True)

    relu = mybir.ActivationFunctionType.Relu
    eq = mybir.AluOpType.is_equal
    mult = mybir.AluOpType.mult
    add = mybir.AluOpType.add

    wins_by_batch = {}
    for (g, h, wins) in layout["win_sched"]:
        wins_by_batch.setdefault(g, []).append((h, wins))

    with tile.TileContext(nc) as tc:
        with (
            tc.tile_pool(name="const", bufs=1) as const,
            tc.tile_pool(name="gbuf", bufs=3) as gbuf,
            tc.tile_pool(name="sell", bufs=8) as sell,
            tc.tile_pool(name="sovf", bufs=16) as sovf,
            tc.tile_pool(name="aggp", bufs=3) as aggp,
            tc.tile_pool(name="hp", bufs=2) as hp,
            tc.tile_pool(name="outp", bufs=3) as outp,
            tc.tile_pool(name="psa", bufs=2, space="PSUM") as psa,
            tc.tile_pool(name="psh", bufs=2, space="PSUM") as psh,
            tc.tile_pool(name="pso", bufs=2, space="PSUM") as pso,
            tc.tile_pool(name="pst", bufs=2, space="PSUM") as pst,
        ):
            c16_s = const.tile([P, C16], bf16)
            nc.sync.dma_start(out=c16_s[:], in_=c16_d[:])
            c32_s = const.tile([P, C32], f32)
            nc.sync.dma_start(out=c32_s[:], in_=c32_d[:])
            gidx_s = const.tile([P, idx_cols], i16)
            nc.sync.dma_start(out=gidx_s[:], in_=gidx_d[:])
            xpermT_s = const.tile([P, tiles_tot * P], bf16)
            nc.sync.dma_start(out=xpermT_s[:], in_=xpermT_d[:])

            iota_s = c16_s[:, O_IOTA:O_IOTA + P]
            pcol_s = c16_s[:, O_PCOL:O_PCOL + 1]

            gq = [0]
            col = [0]

            def emit_tail(g, pagg):
                """Eviction + dense layers + output for batch g (deferred one
                batch so PE/DVE never head-block the next batch's S-gen)."""
                aggT = aggp.tile([P, G_TILES * P], bf16)
                nc.vector.scalar_tensor_tensor(
                    out=aggT[:],
                    in0=pagg[:],
                    scalar=1.0,
                    in1=xpermT_s[:, g * G_TILES * P:(g + 1) * G_TILES * P],
                    op0=mult, op1=add,
                )
                hT = hp.tile([P, 4, G_TILES * P], bf16)
                for cc in range(4):
                    ph = psh.tile([P, G_TILES * P], f32, space="PSUM")
                    nc.tensor.matmul(
                        out=ph[:],
                        lhsT=c16_s[:, O_W1 + cc * P:O_W1 + (cc + 1) * P],
                        rhs=aggT[:], start=True, stop=True)
                    nc.scalar.activation(
                        out=hT[:, cc, :], in_=ph[:], func=relu,
                        bias=c32_s[:, O_B1 + cc:O_B1 + cc + 1], scale=1.0)
                po = pso.tile([P, G_TILES * P], f32, space="PSUM")
                for cc in range(4):
                    nc.tensor.matmul(
                        out=po[:],
                        lhsT=c16_s[:, O_W2 + cc * P:O_W2 + (cc + 1) * P],
                        rhs=hT[:, cc, :], start=(cc == 0), stop=(cc == 3))
                outT = outp.tile([P, G_TILES * P], f32, tag="outT")
                nc.scalar.activation(
                    out=outT[:], in_=po[:], func=relu,
                    bias=c32_s[:, O_B2:O_B2 + 1], scale=1.0)
                nc.sync.dma_start(
                    out=out_d[:, g * G_TILES * P:(g + 1) * G_TILES * P],
                    in_=outT[:])

            prev = None           # (g, pagg) awaiting its deferred tail
            for g in range(n_batches):
                # ---- gathers (pool engine paces the whole kernel) ----
                gb = gbuf.tile([P, slots_max, D_IN], bf16, tag="gb")
                slot = 0
                win_slots = []
                for (h, wins) in wins_by_batch.get(g, []):
                    tab = xtab[0:NA, :] if h == 0 else xtab[NA:NA + NB, :]
                    for n_ch in wins:
                        ni = n_ch * P
                        nc.gpsimd.dma_gather(
                            out_ap=gb[:, slot:slot + n_ch, :],
                            in_ap=tab,
                            idxs_ap=gidx_s[:, col[0]:col[0] + ni // 16],
                            num_idxs=ni, num_idxs_reg=ni,
                            elem_size=D_IN, queue_num=gq[0] % 4,
                            single_packet=True,
                        )
                        gq[0] += 1
                        win_slots.append(slot)
                        slot += n_ch
                        col[0] += ni // 16

                # ---- S generation for this batch (DVE runs ahead) ----
                smap = layout["slot_maps"][g]
                tile_mms = []
                for tb in range(G_TILES):
                    t = g * G_TILES + tb
                    wE, nO = int(ell_w[t]), int(ovf_n[t])
                    mms = []
                    if wE:
                        Se = sell.tile([P, kmax * P], bf16, tag="Se")
                        nc.vector.scalar_tensor_tensor(
                            out=Se[:, 0:wE * P],
                            in0=iota_s.rearrange("p (o c) -> p o c", o=1)
                                      .to_broadcast([P, wE, P]),
                            scalar=pcol_s,
                            in1=c16_s[:, O_WELL[t]:O_WELL[t] + wE]
                                .rearrange("p (k o) -> p k o", o=1)
                                .to_broadcast([P, wE, P]),
                            op0=eq, op1=mult,
                        )
                        for cc in range(wE):
                            mms.append((smap[(t, "ell", cc)],
                                        Se[:, cc * P:(cc + 1) * P]))
                    for cc in range(nO):
                        So = sovf.tile([P, P], bf16, tag="So")
                        ob = O_OVF[t] + 2 * cc
                        nc.vector.scalar_tensor_tensor(
                            out=So[:],
                            in0=iota_s,
                            scalar=c16_s[:, ob + 1:ob + 2],
                            in1=c16_s[:, ob:ob + 1].to_broadcast([P, P]),
                            op0=eq, op1=mult,
                        )
                        mms.append((smap[(t, "ovf", cc)], So[:]))
                    tile_mms.append(mms)

                # ---- previous batch's eviction/dense/output (inputs ready,
                # so PE doesn't stall behind this batch's touches) ----
                if prev is not None:
                    emit_tail(*prev)

                # ---- touches + scatter matmuls ----
                for ws in win_slots:
                    ptouch = pst.tile([P, 1], f32, space="PSUM", tag="pt")
                    nc.tensor.matmul(out=ptouch[0:1, :],
                                     lhsT=gb[:, ws, 0:1],
                                     rhs=gb[:, ws, 0:1],
                                     start=True, stop=True)
                pagg = psa.tile([P, G_TILES * P], f32, space="PSUM")
                for tb in range(G_TILES):
                    mms = tile_mms[tb]
                    for j, (sl, S_ap) in enumerate(mms):
                        nc.tensor.matmul(
                            out=pagg[:, tb * P:(tb + 1) * P],
                            lhsT=gb[:, sl, :],
                            rhs=S_ap,
                            start=(j == 0),
                            stop=(j == len(mms) - 1),
                        )
                    if not mms:
                        nc.vector.memset(pagg[:, tb * P:(tb + 1) * P], 0)
                prev = (g, pagg)

            emit_tail(*prev)

    nc.compile()
    return nc


def _pack_core_inputs(pc, layout, x, dinv, W1, b1, W2, b2, xtab_arr):
    """Build one core's input tensors following the shared schedule."""
    import ml_dtypes
    bf = ml_dtypes.bfloat16

    tiles_tot = layout["tiles_tot"]
    idx_cols = layout["idx_cols"]
    kpad, novpad = layout["kpad"], layout["novpad"]
    O_WELL, O_OVF, C16 = layout["O_WELL"], layout["O_OVF"], layout["C16"]

    # --- cdata16 ---
    c16 = np.zeros((P, C16), np.float32)
    c16[:, 0:P] = np.tile(np.arange(P, dtype=np.float32), (P, 1))
    c16[:, P:P + 1] = np.arange(P, dtype=np.float32)[:, None]
    c16[:, 129:641] = W1
    c16[:, 641:1153] = (W2.reshape(4, P, D_OUT).transpose(1, 0, 2)
                          .reshape(P, 4 * D_OUT))
    for t in range(tiles_tot):
        th = pc["tiles"][t]
        for h in (0, 1):
            kc = th[h]["k"]
            base = O_WELL[t] + (0 if h == 0 else int(kpad[t, 0]))
            if kc:
                c16[:, base:base + kc] = th[h]["ell_w"].T
            ob = O_OVF[t] + 2 * (0 if h == 0 else int(novpad[t, 0]))
            for cc in range(th[h]["nov"]):
                c16[:, ob + 2 * cc] = th[h]["ovf_w"][cc]
                c16[:, ob + 2 * cc + 1] = th[h]["ovf_pos"][cc]
    c16 = np.ascontiguousarray(c16.astype(bf))

    # --- cdata32 ---
    c32 = np.zeros((P, 5), np.float32)
    c32[:, 0:4] = b1.reshape(4, P).T
    c32[:, 4] = b2
    c32 = np.ascontiguousarray(c32)

    # --- gidx stream following win_sched/slot order ---
    # Pad slots must carry a VALID index (HW treats every non-trailing index
    # as an address); forward-fill with the previous slot's index so the
    # duplicate read hits the same HBM row.  Their S value is zero.
    cols = []
    for (g, h, wins) in layout["win_sched"]:
        chunks = []
        for t in range(g * G_TILES, (g + 1) * G_TILES):
            th = pc["tiles"][t][h]
            for cc in range(int(kpad[t, h])):
                chunks.append(th["ell_idx"][cc])
            for cc in range(int(novpad[t, h])):
                chunks.append(th["ovf_idx"][cc])
        assert len(chunks) == sum(wins)
        stream = np.concatenate(chunks)
        bad = stream < 0
        if bad.any():
            idxs = np.where(~bad, np.arange(len(stream)), -1)
            np.maximum.accumulate(idxs, out=idxs)
            stream = np.where(idxs >= 0, stream[np.maximum(idxs, 0)], 0)
        cols.append(stream)
    flat = (np.concatenate(cols) if cols else np.zeros(0, np.int64))
    flat = flat.astype(np.int16)
    g16 = flat.reshape(-1, 16).T.copy()
    g128 = np.tile(g16, (8, 1))
    gidx = np.zeros((P, idx_cols), np.int16)
    gidx[:, 0:g128.shape[1]] = g128

    # --- xpermT: dinv^2 * x rows of own nodes, feature-major ---
    n_per = x.shape[0] // N_CORES
    nodes = np.arange(n_per) + pc["lo"]
    rows = pc["tile_of"].astype(np.int64) * P + pc["pos_in_tile"]
    xpermT = np.zeros((P, tiles_tot * P), np.float32)
    xpermT[:, rows] = (x[nodes] * (dinv[nodes] ** 2)[:, None]).T
    xpermT = np.ascontiguousarray(xpermT.astype(bf))

    return {"xtab": xtab_arr, "xpermT": xpermT, "cdata16": c16,
            "cdata32": c32, "gidx": gidx}


def _install_ntff_hook():
    """The agent image's antenv lacks axon_hooks; fabricate it so trace=True
    can drive NTFF profiling through libaxon_pjrt.so's C ABI."""
    import contextlib
    import ctypes
    import types

    if "antenv.axon_hooks" in sys.modules:
        return
    so_path = "/opt/axon/libaxon_pjrt.so"
    if not os.path.exists(so_path):
        return
    lib = ctypes.CDLL(so_path)
    if not hasattr(lib, "axon_start_nrt_profile"):
        return
    lib.axon_start_nrt_profile.argtypes = [
        ctypes.POINTER(ctypes.c_int64), ctypes.c_size_t]
    lib.axon_start_nrt_profile.restype = ctypes.c_int64
    lib.axon_stop_nrt_profile.argtypes = [ctypes.c_char_p]
    lib.axon_stop_nrt_profile.restype = ctypes.c_int64

    @contextlib.contextmanager
    def _hook(output_dir, device_ids):
        import jax
        jax.devices()
        if device_ids:
            ids = (ctypes.c_int64 * len(device_ids))(*device_ids)
            rc = lib.axon_start_nrt_profile(ids, len(device_ids))
        else:
            rc = lib.axon_start_nrt_profile(None, 0)
        if rc != 0:
            raise RuntimeError(f"axon_start_nrt_profile rc={rc}")
        try:
            yield
        finally:
            n = lib.axon_stop_nrt_profile(str(output_dir).encode())
            print(f"ntff profile: {n} file(s) written to {output_dir}",
                  file=sys.stderr)

    import antenv  # noqa: F401
    mod = types.ModuleType("antenv.axon_hooks")
    mod._hook = _hook
    mod.set_axon_ntff_profile_hook = lambda h: setattr(mod, "_hook", h)
    mod.get_axon_ntff_profile_hook = lambda: mod._hook
    sys.modules["antenv.axon_hooks"] = mod


def _run(nc, in_maps, trace=False):
    if trace:
        try:
            _install_ntff_hook()
        except Exception as e:  # degrade to untraced run
            print(f"ntff hook install failed: {e}", file=sys.stderr)
    from concourse.bass_utils import run_bass_kernel_spmd

    return run_bass_kernel_spmd(
        nc, in_maps, core_ids=list(range(N_CORES)), trace=trace,
    )


def _prepare(x, edge_index, edge_weight, W1, b1, W2, b2):
    import ml_dtypes
    N = x.shape[0]
    per_core, layout, dinv = _preprocess(x, edge_index, edge_weight)
    layout = _schedule(per_core, layout)

    xs = x * dinv[:, None]
    if layout["interleave"]:
        xt = np.empty_like(xs)
        xt[:(N + 1) // 2] = xs[0::2]
        xt[(N + 1) // 2:] = xs[1::2]
    else:
        xt = xs
    xtab_arr = np.ascontiguousarray(xt.astype(ml_dtypes.bfloat16))

    in_maps = [_pack_core_inputs(pc, layout, x, dinv, W1, b1, W2, b2,
                                 xtab_arr) for pc in per_core]
    return per_core, layout, in_maps


def kernel(x, edge_index, edge_weight, W1, b1, W2, b2, _want_trace=False):
    x = np.ascontiguousarray(np.asarray(x, np.float32))
    W1 = np.asarray(W1, np.float32)
    b1 = np.asarray(b1, np.float32)
    W2 = np.asarray(W2, np.float32)
    b2 = np.asarray(b2, np.float32)

    N = x.shape[0]
    per_core, layout, in_maps = _prepare(x, edge_index, edge_weight,
                                         W1, b1, W2, b2)
    nc = _build_program(layout)
    res = _run(nc, in_maps, trace=_want_trace)

    out = np.empty((N, D_IN), np.float32)
    for c in range(N_CORES):
        rows = res.results[c]["out"]          # [128, tiles*P] feature-major
        perm = per_core[c]["perm"]
        valid = perm >= 0
        out[perm[valid]] = rows.T[valid]

    kernel.last_results = res
    return out


# revision 17
# speedup vs baseline: 1.0180x; 1.0180x over previous
"""Trainium2 Bass kernel for a 2-layer GCN (PyG GCNConv + dense layer).

Computation (matches the jax reference):
    deg[n]  = 1 + sum of incoming edge weights        (self loop weight 1)
    dinv    = deg ** -0.5
    norm_e  = dinv[src] * ew * dinv[dst]              (per edge, incl. self)
    agg[n]  = sum_e norm_e * x[src_e]                 (propagate FIRST: A(xW) == (Ax)W)
    h       = relu(agg @ W1 + b1)
    out     = relu(h @ W2 + b2)

Distribution: nodes (as scatter destinations) are partitioned across the 8
cores.  All on-chip compute is bf16 (rel err ~4e-3 vs the 2e-2 gate).

Normalization is folded into host-built tables so the device only does
gather + one-hot matmuls + dense layers:
    xtab[n]   = dinv[n] * x[n]       (bf16 gather table, even nodes first)
    w_e       = ew_e * dinv[dst_e]   (per-edge S value)
    xpermT[n] = dinv[n]^2 * x[n]     (self-loop term, added at eviction)

Per core, nodes are degree-sorted into 128-node dst tiles.  Each tile's
incoming edges are stored ELL-style: ELL chunk c holds the c-th edge of
every dst position, so the tile's S block is a run of diagonals generated
by ONE DVE scalar_tensor_tensor over stride-0 broadcast APs; leftover
high-degree edges go to compact one-hot chunks (iota==mdst S, one STT
each).  Gathered rows are node-major (dma_gather from HBM in bf16, 256B
descriptors; -1 pad indices skip their descriptor), and each 128-edge
chunk feeds  lhsT^T @ S  accumulated feature-major in PSUM.  Self loops
are never gathered: the xpermT table is added during the PSUM->SBUF
eviction (scalar_tensor_tensor add).  Dense W1/W2 run feature-major in
bf16; the output stays feature-major and contiguous, and the host does
the final transpose + row un-permutation.

Known HW constraints baked in: dma_gather max 1024 indices per
single-packet instruction; gathers rotate over 4 SWDGE queues; per-gather
touch matmuls keep S3_LW matmuls at <=2 sync waits; int16 gather indices
cap table views at 32768 rows (even/odd interleave keeps the two views
balanced); SPMD means one program serves all 8 cores, so chunk counts are
padded to the per-tile max across cores (pad idx = -1 -> descriptor
skipped, pad S value = 0).
"""

import os
import sys

import numpy as np

sys.path.insert(0, "/opt/trn_rl_repo")

P = 128
N_CORES = 8
HALF = 32768          # int16 index limit per gather table view
G_TILES = 4           # dst tiles per batch (one PSUM [128, 512] bank)
GMAX = 8              # chunks per gather instruction (1024 idx, 1 packet)

D_IN = 128
D_HID = 512
D_OUT = 128


def _best_k_shared(deg_list):
    """ELL depth K for one (tile, half) shared by all cores.  An ELL chunk
    costs a matmul + amortized slice of one wide STT + pad-slot DMA dups; an
    overflow chunk costs a matmul + its own full STT + DMA.  Minimize
    2*K + 3*max_core(ceil(overflow_c(K)/128)), ties toward smaller K."""
    dmax = max(int(d.max(initial=0)) for d in deg_list)
    if dmax == 0:
        return 0
    best = None
    for k in range(0, dmax + 1):
        novm = max(-(-int(np.maximum(d - k, 0).sum()) // P) for d in deg_list)
        key = (2 * k + 3 * novm, k)
        if best is None or key < best[0]:
            best = (key, k)
    return best[1]


def _preprocess(x, edge_index, edge_weight):
    """Graph preprocessing; per-core raw ELL/overflow structures."""
    N = x.shape[0]
    n_per = N // N_CORES
    assert n_per * N_CORES == N

    src = np.asarray(edge_index[0], np.int64)
    dst = np.asarray(edge_index[1], np.int64)
    ew = np.asarray(edge_weight, np.float32)
    ids = np.arange(N, dtype=np.int64)
    dst_f = np.concatenate([dst, ids])
    ew_f = np.concatenate([ew, np.ones(N, np.float32)])

    deg = np.bincount(dst_f, weights=ew_f.astype(np.float64), minlength=N)
    deg = deg.astype(np.float32)
    dinv = np.where(deg > 0, 1.0 / np.sqrt(deg), 0.0).astype(np.float32)

    # self loops are handled by the xpermT table; only real edges get slots
    w_all = (ew * dinv[dst]).astype(np.float32)

    interleave = N > HALF
    rows_a = (N + 1) // 2 if interleave else N

    n_tiles = -(-n_per // P)
    n_batches = -(-n_tiles // G_TILES)
    tiles_tot = n_batches * G_TILES

    per_core = []
    for c in range(N_CORES):
        lo = c * n_per
        m = (dst >= lo) & (dst < lo + n_per)
        es, ed, wc = src[m], (dst[m] - lo).astype(np.int64), w_all[m]
        if interleave:
            e_half = (es % 2).astype(np.int64)
            e_idx = (es // 2).astype(np.int64)
        else:
            e_half = np.zeros(len(es), np.int64)
            e_idx = es

        # degree-sorted dst tiles: uniform per-tile degree -> tight ELL.
        # Degree-rank groups are dealt round-robin across batches so every
        # batch carries a balanced chunk load (rank r -> batch r % n_batches).
        degl = np.bincount(ed, minlength=n_per)
        order_nodes = np.argsort(-degl, kind="stable")
        rank = np.arange(n_per) // P
        remap = np.empty(n_tiles, np.int64)
        for r in range(n_tiles):
            remap[r] = (r % n_batches) * G_TILES + (r // n_batches)
        tile_of = np.empty(n_per, np.int32)
        pos_in_tile = np.empty(n_per, np.int32)
        tile_of[order_nodes] = remap[rank]
        pos_in_tile[order_nodes] = np.arange(n_per) % P

        te = tile_of[ed]
        pe_ = pos_in_tile[ed]

        tiles = []
        for t in range(tiles_tot):
            th = {}
            for h in (0, 1):
                sel = (te == t) & (e_half == h)
                eposs, eidxs, ws = pe_[sel], e_idx[sel], wc[sel]
                o = np.lexsort((np.arange(len(eposs)), eposs))
                th[h] = dict(
                    degs=np.bincount(eposs, minlength=P),
                    eidxs=eidxs[o], ws=ws[o],
                )
            tiles.append(th)

        perm = np.full(tiles_tot * P, -1, np.int64)
        node_rows = tile_of.astype(np.int64) * P + pos_in_tile
        perm[node_rows] = np.arange(n_per) + lo

        per_core.append(dict(tiles=tiles, perm=perm, lo=lo,
                             tile_of=tile_of, pos_in_tile=pos_in_tile))

    layout = dict(
        n_batches=n_batches, tiles_tot=tiles_tot, n_tiles_real=n_tiles,
        n_rows_A=rows_a, n_rows_B=(N - rows_a) if interleave else 0,
        interleave=interleave,
    )
    return per_core, layout, dinv


def _build_ell(th, k, nov):
    """Materialize one (tile, half)'s ELL block at depth k plus nov overflow
    chunks.  Pad indices MUST stay valid (the HW gather treats every
    non-trailing index as an address); use the previous slot's index."""
    degs, eidxs, ws = th["degs"], th["eidxs"], th["ws"]
    starts = np.concatenate([[0], np.cumsum(degs)])
    ell_idx = np.full((k, P), -1, np.int64)
    ell_w = np.zeros((k, P), np.float32)
    ovf_i, ovf_p, ovf_w = [], [], []
    for p in range(P):
        s0, d = int(starts[p]), int(degs[p])
        take = min(d, k)
        ell_idx[:take, p] = eidxs[s0:s0 + take]
        ell_w[:take, p] = ws[s0:s0 + take]
        if d > k:
            ovf_i.append(eidxs[s0 + take:s0 + d])
            ovf_p.append(np.full(d - k, p, np.int64))
            ovf_w.append(ws[s0 + take:s0 + d])
    ovf_i = np.concatenate(ovf_i) if ovf_i else np.empty(0, np.int64)
    ovf_p = np.concatenate(ovf_p) if ovf_p else np.empty(0, np.int64)
    ovf_w = np.concatenate(ovf_w) if ovf_w else np.empty(0, np.float32)
    assert len(ovf_i) <= nov * P
    pad = nov * P - len(ovf_i)
    th["k"], th["nov"] = k, nov
    th["ell_idx"] = ell_idx
    th["ell_w"] = ell_w
    th["ovf_idx"] = np.concatenate(
        [ovf_i, np.full(pad, -1, np.int64)]).reshape(nov, P)
    th["ovf_pos"] = np.concatenate(
        [ovf_p, np.zeros(pad, np.int64)]).reshape(nov, P)
    th["ovf_w"] = np.concatenate(
        [ovf_w, np.zeros(pad, np.float32)]).reshape(nov, P)


def _schedule(per_core, layout):
    """Shared chunk/window schedule: every (tile, half) uses one ELL depth
    across all cores (chosen to minimize the shared padded chunk count), and
    overflow chunk counts are padded to the max across cores."""
    n_batches = layout["n_batches"]
    tiles_tot = layout["tiles_tot"]

    kpad = np.zeros((tiles_tot, 2), np.int64)
    novpad = np.zeros((tiles_tot, 2), np.int64)
    for t in range(tiles_tot):
        for h in (0, 1):
            k = _best_k_shared([pc["tiles"][t][h]["degs"] for pc in per_core])
            kpad[t, h] = k
            nov = 0
            for pc in per_core:
                degs = pc["tiles"][t][h]["degs"]
                nov = max(nov, -(-int(np.maximum(degs - k, 0).sum()) // P))
            novpad[t, h] = nov
            for pc in per_core:
                _build_ell(pc["tiles"][t][h], k, nov)
    ell_w = kpad.sum(axis=1)            # S_ell width per tile
    ovf_n = novpad.sum(axis=1)
    layout["kpad"] = kpad
    layout["novpad"] = novpad
    layout["ell_w"] = ell_w
    layout["ovf_n"] = ovf_n
    layout["kmax"] = int(max(1, ell_w.max()))

    # slot map per batch + window schedule; chunk stream per (batch, view):
    # for each tile of the batch: ELL chunks then ovf chunks
    win_sched = []          # (batch, view, [chunks per window])
    slot_maps = []          # per batch: {(t, "ell"/"ovf", combined_idx): slot}
    slots_max = 1
    for g in range(n_batches):
        smap = {}
        slot = 0
        for h in (0, 1):
            h0 = slot
            for t in range(g * G_TILES, (g + 1) * G_TILES):
                base_e = 0 if h == 0 else kpad[t, 0]
                base_o = 0 if h == 0 else novpad[t, 0]
                for cc in range(kpad[t, h]):
                    smap[(t, "ell", base_e + cc)] = slot
                    slot += 1
                for cc in range(novpad[t, h]):
                    smap[(t, "ovf", base_o + cc)] = slot
                    slot += 1
            n_ch = slot - h0
            wins = []
            while n_ch > 0:
                take = min(GMAX, n_ch)
                wins.append(take)
                n_ch -= take
            if wins:
                win_sched.append((g, h, wins))
        slot_maps.append(smap)
        slots_max = max(slots_max, slot)
    layout["win_sched"] = win_sched
    layout["slot_maps"] = slot_maps
    layout["slots_max"] = slots_max
    total_idx = sum(sum(w) for (_, _, ws) in win_sched for w in [ws]) * P
    layout["idx_cols"] = max(8, total_idx // 16)

    # cdata16 layout: iota(128) | pcol(1) | w1(512) | w2r(512) |
    #                 wELL blocks | (w, mdst) pairs per ovf chunk
    off = 1153
    O_WELL = []
    for t in range(tiles_tot):
        O_WELL.append(off)
        off += int(ell_w[t])
    O_OVF = []
    for t in range(tiles_tot):
        O_OVF.append(off)
        off += 2 * int(ovf_n[t])
    layout["O_WELL"] = O_WELL
    layout["O_OVF"] = O_OVF
    layout["C16"] = off
    return layout


def _build_program(layout):
    from concourse import bacc, mybir, tile

    f32 = mybir.dt.float32
    bf16 = mybir.dt.bfloat16
    i16 = mybir.dt.int16

    n_batches = layout["n_batches"]
    tiles_tot = layout["tiles_tot"]
    slots_max = layout["slots_max"]
    idx_cols = layout["idx_cols"]
    NA, NB = layout["n_rows_A"], layout["n_rows_B"]
    kmax = layout["kmax"]
    C16 = layout["C16"]
    O_WELL, O_OVF = layout["O_WELL"], layout["O_OVF"]
    ell_w, ovf_n = layout["ell_w"], layout["ovf_n"]
    O_IOTA, O_PCOL, O_W1, O_W2 = 0, 128, 129, 641
    O_B1, O_B2, C32 = 0, 4, 5

    # Gather wall: each 1024-idx dma_gather costs ~8.6us of Q7 exec on its
    # queue's cpu pair; 4 queues run pairs concurrently -> ~2.2us/instr
    # steady state.  Everything else must hide under that.
    nc = bacc.Bacc("TRN2", num_swdge_queues=4)
    xtab = nc.declare_dram_parameter("xtab", [NA + NB, D_IN], bf16,
                                     isOutput=False)
    xpermT_d = nc.declare_dram_parameter("xpermT", [P, tiles_tot * P], bf16,
                                         isOutput=False)
    c16_d = nc.declare_dram_parameter("cdata16", [P, C16], bf16,
                                      isOutput=False)
    c32_d = nc.declare_dram_parameter("cdata32", [P, C32], f32,
                                      isOutput=False)
    gidx_d = nc.declare_dram_parameter("gidx", [P, idx_cols], i16,
                                       isOutput=False)
    out_d = nc.declare_dram_parameter("out", [P, tiles_tot * P], f32,
                                      isOutput=True)

    relu = mybir.ActivationFunctionType.Relu
    eq = mybir.AluOpType.is_equal
    mult = mybir.AluOpType.mult
    add = mybir.AluOpType.add

    wins_by_batch = {}
    for (g, h, wins) in layout["win_sched"]:
        wins_by_batch.setdefault(g, []).append((h, wins))

    with tile.TileContext(nc) as tc:
        with (
            tc.tile_pool(name="const", bufs=1) as const,
            tc.tile_pool(name="gbuf", bufs=3) as gbuf,
            tc.tile_pool(name="sell", bufs=8) as sell,
            tc.tile_pool(name="sovf", bufs=16) as sovf,
            tc.tile_pool(name="aggp", bufs=3) as aggp,
            tc.tile_pool(name="hp", bufs=2) as hp,
            tc.tile_pool(name="outp", bufs=3) as outp,
            tc.tile_pool(name="psa", bufs=2, space="PSUM") as psa,
            tc.tile_pool(name="psh", bufs=2, space="PSUM") as psh,
            tc.tile_pool(name="pso", bufs=2, space="PSUM") as pso,
            tc.tile_pool(name="pst", bufs=2, space="PSUM") as pst,
        ):
            gidx_s = const.tile([P, idx_cols], i16)
            nc.sync.dma_start(out=gidx_s[:], in_=gidx_d[:])
            c16_s = const.tile([P, C16], bf16)
            nc.sync.dma_start(out=c16_s[:], in_=c16_d[:])
            c32_s = const.tile([P, C32], f32)
            nc.sync.dma_start(out=c32_s[:], in_=c32_d[:])
            xpermT_s = const.tile([P, tiles_tot * P], bf16)
            nc.sync.dma_start(out=xpermT_s[:], in_=xpermT_d[:])

            iota_s = c16_s[:, O_IOTA:O_IOTA + P]
            pcol_s = c16_s[:, O_PCOL:O_PCOL + 1]

            gq = [0]
            col = [0]

            def emit_tail(g, pagg):
                """Eviction + dense layers + output for batch g (deferred one
                batch so PE/DVE never head-block the next batch's S-gen)."""
                aggT = aggp.tile([P, G_TILES * P], bf16)
                nc.vector.scalar_tensor_tensor(
                    out=aggT[:],
                    in0=pagg[:],
                    scalar=1.0,
                    in1=xpermT_s[:, g * G_TILES * P:(g + 1) * G_TILES * P],
                    op0=mult, op1=add,
                )
                hT = hp.tile([P, 4, G_TILES * P], bf16)
                for cc in range(4):
                    ph = psh.tile([P, G_TILES * P], f32, space="PSUM")
                    nc.tensor.matmul(
                        out=ph[:],
                        lhsT=c16_s[:, O_W1 + cc * P:O_W1 + (cc + 1) * P],
                        rhs=aggT[:], start=True, stop=True)
                    nc.scalar.activation(
                        out=hT[:, cc, :], in_=ph[:], func=relu,
                        bias=c32_s[:, O_B1 + cc:O_B1 + cc + 1], scale=1.0)
                po = pso.tile([P, G_TILES * P], f32, space="PSUM")
                for cc in range(4):
                    nc.tensor.matmul(
                        out=po[:],
                        lhsT=c16_s[:, O_W2 + cc * P:O_W2 + (cc + 1) * P],
                        rhs=hT[:, cc, :], start=(cc == 0), stop=(cc == 3))
                outT = outp.tile([P, G_TILES * P], f32, tag="outT")
                nc.scalar.activation(
                    out=outT[:], in_=po[:], func=relu,
                    bias=c32_s[:, O_B2:O_B2 + 1], scale=1.0)
                nc.sync.dma_start(
                    out=out_d[:, g * G_TILES * P:(g + 1) * G_TILES * P],
                    in_=outT[:])

            prev = None           # (g, pagg) awaiting its deferred tail
            for g in range(n_batches):
                # ---- gathers (pool engine paces the whole kernel) ----
                gb = gbuf.tile([P, slots_max, D_IN], bf16, tag="gb")
                slot = 0
                win_slots = []
                for (h, wins) in wins_by_batch.get(g, []):
                    tab = xtab[0:NA, :] if h == 0 else xtab[NA:NA + NB, :]
                    for n_ch in wins:
                        ni = n_ch * P
                        nc.gpsimd.dma_gather(
                            out_ap=gb[:, slot:slot + n_ch, :],
                            in_ap=tab,
                            idxs_ap=gidx_s[:, col[0]:col[0] + ni // 16],
                            num_idxs=ni, num_idxs_reg=ni,
                            elem_size=D_IN, queue_num=gq[0] % 4,
                            single_packet=True,
                        )
                        gq[0] += 1
                        win_slots.append(slot)
                        slot += n_ch
                        col[0] += ni // 16

                # ---- previous batch's eviction/dense/output FIRST: its
                # inputs are ready, so neither DVE nor PE head-block on the
                # next batch's work ----
                if prev is not None:
                    emit_tail(*prev)
                    prev = None

                # ---- S generation for this batch (DVE runs ahead) ----
                smap = layout["slot_maps"][g]
                tile_mms = []
                for tb in range(G_TILES):
                    t = g * G_TILES + tb
                    wE, nO = int(ell_w[t]), int(ovf_n[t])
                    mms = []
                    if wE:
                        Se = sell.tile([P, kmax * P], bf16, tag="Se")
                        nc.vector.scalar_tensor_tensor(
                            out=Se[:, 0:wE * P],
                            in0=iota_s.rearrange("p (o c) -> p o c", o=1)
                                      .to_broadcast([P, wE, P]),
                            scalar=pcol_s,
                            in1=c16_s[:, O_WELL[t]:O_WELL[t] + wE]
                                .rearrange("p (k o) -> p k o", o=1)
                                .to_broadcast([P, wE, P]),
                            op0=eq, op1=mult,
                        )
                        for cc in range(wE):
                            mms.append((smap[(t, "ell", cc)],
                                        Se[:, cc * P:(cc + 1) * P]))
                    for cc in range(nO):
                        So = sovf.tile([P, P], bf16, tag="So")
                        ob = O_OVF[t] + 2 * cc
                        nc.vector.scalar_tensor_tensor(
                            out=So[:],
                            in0=iota_s,
                            scalar=c16_s[:, ob + 1:ob + 2],
                            in1=c16_s[:, ob:ob + 1].to_broadcast([P, P]),
                            op0=eq, op1=mult,
                        )
                        mms.append((smap[(t, "ovf", cc)], So[:]))
                    tile_mms.append(mms)

                # ---- touches + scatter matmuls (one touch covers two
                # windows via its two operands: <=2 sem waits per matmul) ----
                for wi in range(0, len(win_slots), 2):
                    wa = win_slots[wi]
                    wb = win_slots[min(wi + 1, len(win_slots) - 1)]
                    ptouch = pst.tile([P, 1], f32, space="PSUM", tag="pt")
                    nc.tensor.matmul(out=ptouch[0:1, :],
                                     lhsT=gb[:, wa, 0:1],
                                     rhs=gb[:, wb, 0:1],
                                     start=True, stop=True)
                pagg = psa.tile([P, G_TILES * P], f32, space="PSUM")
                for tb in range(G_TILES):
                    mms = tile_mms[tb]
                    for j, (sl, S_ap) in enumerate(mms):
                        nc.tensor.matmul(
                            out=pagg[:, tb * P:(tb + 1) * P],
                            lhsT=gb[:, sl, :],
                            rhs=S_ap,
                            start=(j == 0),
                            stop=(j == len(mms) - 1),
                        )
                    if not mms:
                        nc.vector.memset(pagg[:, tb * P:(tb + 1) * P], 0)
                prev = (g, pagg)

            emit_tail(*prev)

    nc.compile()
    return nc


def _pack_core_inputs(pc, layout, x, dinv, W1, b1, W2, b2, xtab_arr):
    """Build one core's input tensors following the shared schedule."""
    import ml_dtypes
    bf = ml_dtypes.bfloat16

    tiles_tot = layout["tiles_tot"]
    idx_cols = layout["idx_cols"]
    kpad, novpad = layout["kpad"], layout["novpad"]
    O_WELL, O_OVF, C16 = layout["O_WELL"], layout["O_OVF"], layout["C16"]

    # --- cdata16 ---
    c16 = np.zeros((P, C16), np.float32)
    c16[:, 0:P] = np.tile(np.arange(P, dtype=np.float32), (P, 1))
    c16[:, P:P + 1] = np.arange(P, dtype=np.float32)[:, None]
    c16[:, 129:641] = W1
    c16[:, 641:1153] = (W2.reshape(4, P, D_OUT).transpose(1, 0, 2)
                          .reshape(P, 4 * D_OUT))
    for t in range(tiles_tot):
        th = pc["tiles"][t]
        for h in (0, 1):
            kc = th[h]["k"]
            base = O_WELL[t] + (0 if h == 0 else int(kpad[t, 0]))
            if kc:
                c16[:, base:base + kc] = th[h]["ell_w"].T
            ob = O_OVF[t] + 2 * (0 if h == 0 else int(novpad[t, 0]))
            for cc in range(th[h]["nov"]):
                c16[:, ob + 2 * cc] = th[h]["ovf_w"][cc]
                c16[:, ob + 2 * cc + 1] = th[h]["ovf_pos"][cc]
    c16 = np.ascontiguousarray(c16.astype(bf))

    # --- cdata32 ---
    c32 = np.zeros((P, 5), np.float32)
    c32[:, 0:4] = b1.reshape(4, P).T
    c32[:, 4] = b2
    c32 = np.ascontiguousarray(c32)

    # --- gidx stream following win_sched/slot order ---
    # Pad slots must carry a VALID index (HW treats every non-trailing index
    # as an address); forward-fill with the previous slot's index so the
    # duplicate read hits the same HBM row.  Their S value is zero.
    cols = []
    for (g, h, wins) in layout["win_sched"]:
        chunks = []
        for t in range(g * G_TILES, (g + 1) * G_TILES):
            th = pc["tiles"][t][h]
            for cc in range(int(kpad[t, h])):
                chunks.append(th["ell_idx"][cc])
            for cc in range(int(novpad[t, h])):
                chunks.append(th["ovf_idx"][cc])
        assert len(chunks) == sum(wins)
        stream = np.concatenate(chunks)
        bad = stream < 0
        if bad.any():
            idxs = np.where(~bad, np.arange(len(stream)), -1)
            np.maximum.accumulate(idxs, out=idxs)
            stream = np.where(idxs >= 0, stream[np.maximum(idxs, 0)], 0)
        cols.append(stream)
    flat = (np.concatenate(cols) if cols else np.zeros(0, np.int64))
    flat = flat.astype(np.int16)
    g16 = flat.reshape(-1, 16).T.copy()
    g128 = np.tile(g16, (8, 1))
    gidx = np.zeros((P, idx_cols), np.int16)
    gidx[:, 0:g128.shape[1]] = g128

    # --- xpermT: dinv^2 * x rows of own nodes, feature-major ---
    n_per = x.shape[0] // N_CORES
    nodes = np.arange(n_per) + pc["lo"]
    rows = pc["tile_of"].astype(np.int64) * P + pc["pos_in_tile"]
    xpermT = np.zeros((P, tiles_tot * P), np.float32)
    xpermT[:, rows] = (x[nodes] * (dinv[nodes] ** 2)[:, None]).T
    xpermT = np.ascontiguousarray(xpermT.astype(bf))

    return {"xtab": xtab_arr, "xpermT": xpermT, "cdata16": c16,
            "cdata32": c32, "gidx": gidx}


def _install_ntff_hook():
    """The agent image's antenv lacks axon_hooks; fabricate it so trace=True
    can drive NTFF profiling through libaxon_pjrt.so's C ABI."""
    import contextlib
    import ctypes
    import types

    if "antenv.axon_hooks" in sys.modules:
        return
    so_path = "/opt/axon/libaxon_pjrt.so"
    if not os.path.exists(so_path):
        return
    lib = ctypes.CDLL(so_path)
    if not hasattr(lib, "axon_start_nrt_profile"):
        return
    lib.axon_start_nrt_profile.argtypes = [
        ctypes.POINTER(ctypes.c_int64), ctypes.c_size_t]
    lib.axon_start_nrt_profile.restype = ctypes.c_int64
    lib.axon_stop_nrt_profile.argtypes = [ctypes.c_char_p]
    lib.axon_stop_nrt_profile.restype = ctypes.c_int64

    @contextlib.contextmanager
    def _hook(output_dir, device_ids):
        import jax
        jax.devices()
        if device_ids:
            ids = (ctypes.c_int64 * len(device_ids))(*device_ids)
            rc = lib.axon_start_nrt_profile(ids, len(device_ids))
        else:
            rc = lib.axon_start_nrt_profile(None, 0)
        if rc != 0:
            raise RuntimeError(f"axon_start_nrt_profile rc={rc}")
        try:
            yield
        finally:
            n = lib.axon_stop_nrt_profile(str(output_dir).encode())
            print(f"ntff profile: {n} file(s) written to {output_dir}",
                  file=sys.stderr)

    import antenv  # noqa: F401
    mod = types.ModuleType("antenv.axon_hooks")
    mod._hook = _hook
    mod.set_axon_ntff_profile_hook = lambda h: setattr(mod, "_hook", h)
    mod.get_axon_ntff_profile_hook = lambda: mod._hook
    sys.modules["antenv.axon_hooks"] = mod


def _run(nc, in_maps, trace=False):
    if trace:
        try:
            _install_ntff_hook()
        except Exception as e:  # degrade to untraced run
            print(f"ntff hook install failed: {e}", file=sys.stderr)
    from concourse.bass_utils import run_bass_kernel_spmd

    return run_bass_kernel_spmd(
        nc, in_maps, core_ids=list(range(N_CORES)), trace=trace,
    )


def _prepare(x, edge_index, edge_weight, W1, b1, W2, b2):
    import ml_dtypes
    N = x.shape[0]
    per_core, layout, dinv = _preprocess(x, edge_index, edge_weight)
    layout = _schedule(per_core, layout)

    xs = x * dinv[:, None]
    if layout["interleave"]:
        xt = np.empty_like(xs)
        xt[:(N + 1) // 2] = xs[0::2]
        xt[(N + 1) // 2:] = xs[1::2]
    else:
        xt = xs
    xtab_arr = np.ascontiguousarray(xt.astype(ml_dtypes.bfloat16))

    in_maps = [_pack_core_inputs(pc, layout, x, dinv, W1, b1, W2, b2,
                                 xtab_arr) for pc in per_core]
    return per_core, layout, in_maps


def kernel(x, edge_index, edge_weight, W1, b1, W2, b2, _want_trace=False):
    x = np.ascontiguousarray(np.asarray(x, np.float32))
    W1 = np.asarray(W1, np.float32)
    b1 = np.asarray(b1, np.float32)
    W2 = np.asarray(W2, np.float32)
    b2 = np.asarray(b2, np.float32)

    N = x.shape[0]
    per_core, layout, in_maps = _prepare(x, edge_index, edge_weight,
                                         W1, b1, W2, b2)
    nc = _build_program(layout)
    res = _run(nc, in_maps, trace=_want_trace)

    out = np.empty((N, D_IN), np.float32)
    for c in range(N_CORES):
        rows = res.results[c]["out"]          # [128, tiles*P] feature-major
        perm = per_core[c]["perm"]
        valid = perm >= 0
        out[perm[valid]] = rows.T[valid]

    kernel.last_results = res
    return out
